# revision 6
# baseline (speedup 1.0000x reference)
"""Embedding lookup (weight[indices]) on 8 TRN2 NeuronCores.

Strategy: replicate the 1M x 128 table in each core's HBM (cast to bf16
host-side; rel err ~2^-9 << 2e-2 tolerance), shard the 4096*200 = 819200
indices 8 ways (data parallel).

The SWDGE indirect-DMA path costs ~1 us of Pool-engine descriptor-gen
per instruction and HW only honours ONE offset per partition per
instruction (measured), so gathering row-by-row is fixed-cost bound.
Instead each core uses the InstDMAGatherAnt ucode gather (`dma_gather`),
which takes thousands of int16 indices per instruction: the host sorts
each core's indices, buckets them into 31 table chunks of <=32768 rows
(so low 15 bits fit int16), pads each bucket to a static M with dummy
index 0, and builds the 16-partition-wrapped replicated index tensor.
The device runs one dma_gather + one contiguous store per chunk; the
host inverts the sort/bucket permutation on the bf16 output and upcasts
to f32.  Sorting also makes the gather's HBM reads near-sequential.
"""

import numpy as np
import ml_dtypes

NUM_EMB = 1_000_000
D = 128
N_CORES = 8
P = 128

N_CHUNKS = 31
CHUNK = -(-NUM_EMB // N_CHUNKS)  # 32259 <= 32768 so low bits fit int16
M_FLOOR = 3584                   # per-chunk padded capacity floor (28 slots)

# tuning knobs
BUFS = 4          # SBUF tile double-buffering depth
DTYPE = "bf16"    # "bf16" or "f32" table/output dtype on device
N_QUEUES = 1      # SWDGE queues (1..4); sub-gathers round-robin over them
M_SUB = 1024      # max indices per dma_gather (ucode descriptor-ring limit)

_CACHE = {}


def host_prep(idx_core: np.ndarray, m: int):
    """Bucket+sort one core's indices. Returns (idx16 [P, N_CHUNKS*m//16],
    dram_row [n] inverse map into the gout tensor)."""
    n = idx_core.shape[0]
    order = np.argsort(idx_core, kind="stable")  # sorts by (chunk, low)
    sidx = idx_core[order]
    chunk = sidx // CHUNK
    low = (sidx - chunk * CHUNK).astype(np.int16)
    counts = np.bincount(chunk, minlength=N_CHUNKS)
    starts = np.concatenate([[0], np.cumsum(counts)[:-1]])
    within = np.arange(n) - starts[chunk]          # slot j within bucket
    idx16 = np.zeros((N_CHUNKS, m), dtype=np.int16)  # pad = dummy row 0
    idx16[chunk, within] = low
    s = m // P
    dram_row = np.empty(n, dtype=np.int64)
    dram_row[order] = chunk * m + (within % P) * s + within // P
    # wrap: index i at partition i%16, position i//16 (per chunk), then
    # replicate the 16-partition block to 128 partitions
    wrapped = (
        idx16.reshape(N_CHUNKS, m // 16, 16)
        .transpose(2, 0, 1)
        .reshape(16, N_CHUNKS * (m // 16))
    )
    return np.tile(wrapped, (8, 1)), dram_row


def _build_bass(m: int, bufs: int, dtype: str, n_queues: int, m_sub: int = M_SUB):
    import concourse.bacc as bacc
    import concourse.mybir as mybir
    import concourse.tile as tile
    from concourse import library_config

    key = (m, bufs, dtype, n_queues, m_sub)
    if key in _CACHE:
        return _CACHE[key]

    bdt = mybir.dt.bfloat16 if dtype == "bf16" else mybir.dt.float32
    s = m // P

    nc = bacc.Bacc(
        "TRN2",
        target_bir_lowering=False,
        debug=False,
        num_devices=N_CORES,
        num_swdge_queues=n_queues,
    )
    idx16_d = nc.dram_tensor(
        "idx16", [P, N_CHUNKS * (m // 16)], mybir.dt.int16, kind="ExternalInput"
    )
    weight = nc.dram_tensor("weight", [NUM_EMB, D], bdt, kind="ExternalInput")
    gout = nc.dram_tensor("gout", [N_CHUNKS * m, D], bdt, kind="ExternalOutput")

    with tile.TileContext(nc) as tc:
        with (
            tc.tile_pool(name="idxp", bufs=1) as idxp,
            tc.tile_pool(name="data", bufs=bufs) as datap,
        ):
            nc.gpsimd.load_library(library_config.mlp)
            idx_tile = idxp.tile([P, N_CHUNKS * (m // 16)], mybir.dt.int16)
            nc.sync.dma_start(idx_tile[:], idx16_d[:])
            gout_r = gout[:].rearrange("(c p s) d -> c p (s d)", c=N_CHUNKS, p=P)
            q = 0
            for c in range(N_CHUNKS):
                hi = min((c + 1) * CHUNK, NUM_EMB)
                dtile = datap.tile([P, s, D], bdt)
                # descriptor-ring limit: <= m_sub indices per dma_gather;
                # sub-gather g fills slots [g*m_sub/P, ...) of the tile, and
                # index i of sub-gather g lands at dtile[i%P][g*m_sub/P+i/P]
                # == absolute slot j//P for j = g*m_sub+i, same as one big
                # gather would
                for g in range(0, m, m_sub):
                    n_sub = min(m_sub, m - g)
                    nc.gpsimd.dma_gather(
                        dtile[:, g // P : (g + n_sub) // P, :],
                        weight[c * CHUNK : hi],
                        idx_tile[:, (c * m + g) // 16 : (c * m + g + n_sub) // 16],
                        n_sub,
                        n_sub,
                        D,
                        queue_num=q % n_queues,
                    )
                    q += 1
                nc.sync.dma_start(gout_r[c], dtile[:].rearrange("p s d -> p (s d)"))
    nc.compile()
    _CACHE[key] = nc
    return nc


def run_sharded(indices: np.ndarray, weight: np.ndarray, trace: bool = False):
    """Shard indices across 8 cores, run the Bass kernel, return
    (full_output, BassKernelResults)."""
    from concourse.bass_utils import run_bass_kernel_spmd

    idx_flat = np.ascontiguousarray(indices.reshape(-1).astype(np.int64))
    if DTYPE == "bf16":
        w = np.ascontiguousarray(weight.astype(ml_dtypes.bfloat16))
    else:
        w = np.ascontiguousarray(weight, dtype=np.float32)
    n_idx = idx_flat.shape[0]
    per_core = n_idx // N_CORES
    assert n_idx == per_core * N_CORES

    # static padded bucket capacity shared by all cores (SPMD): max count
    # over all (core, chunk) buckets, rounded up to a full 128-slot multiple
    chunk_ids = idx_flat // CHUNK
    max_count = max(
        int(np.bincount(chunk_ids[c * per_core : (c + 1) * per_core],
                        minlength=N_CHUNKS).max())
        for c in range(N_CORES)
    )
    m = max(M_FLOOR, -(-max_count // P) * P)

    preps = [
        host_prep(idx_flat[c * per_core : (c + 1) * per_core], m)
        for c in range(N_CORES)
    ]

    nc = _build_bass(m, BUFS, DTYPE, N_QUEUES)
    in_maps = [{"idx16": preps[c][0], "weight": w} for c in range(N_CORES)]
    res = run_bass_kernel_spmd(
        nc, in_maps, core_ids=list(range(N_CORES)), trace=trace
    )
    full = np.empty((n_idx, D), dtype=np.float32)
    for c in range(N_CORES):
        gout = np.asarray(res.results[c]["gout"])
        full[c * per_core : (c + 1) * per_core] = gout[preps[c][1]].astype(np.float32)
    return full.reshape(indices.shape + (D,)), res


def kernel(indices: np.ndarray, weight: np.ndarray) -> np.ndarray:
    full, _ = run_sharded(indices, weight, trace=False)
    return full


# revision 9
# speedup vs baseline: 3.1278x; 3.1278x over previous
"""Embedding lookup (weight[indices]) on 8 TRN2 NeuronCores.

Strategy: replicate the 1M x 128 table in each core's HBM (cast to bf16
host-side; rel err ~2^-9 << 2e-2 tolerance), shard the 4096*200 = 819200
indices 8 ways (data parallel).

The SWDGE indirect-DMA path costs ~1 us of Pool-engine descriptor-gen
per instruction and HW only honours ONE offset per partition per
instruction (measured), so gathering row-by-row is fixed-cost bound.
Instead each core uses the InstDMAGatherAnt ucode gather (`dma_gather`),
which takes thousands of int16 indices per instruction: the host sorts
each core's indices, buckets them into 31 table chunks of <=32768 rows
(so low 15 bits fit int16), pads each bucket to a static M with dummy
index 0, and builds the 16-partition-wrapped replicated index tensor.
The device runs one dma_gather + one contiguous store per chunk; the
host inverts the sort/bucket permutation on the bf16 output and upcasts
to f32.  Sorting also makes the gather's HBM reads near-sequential.
"""

import numpy as np
import ml_dtypes

NUM_EMB = 1_000_000
D = 128
N_CORES = 8
P = 128

N_CHUNKS = 31
CHUNK = -(-NUM_EMB // N_CHUNKS)  # 32259 <= 32768 so low bits fit int16
M_FLOOR = 3584                   # per-chunk padded capacity floor (28 slots)

# tuning knobs
BUFS = 4          # SBUF tile double-buffering depth
DTYPE = "bf16"    # "bf16" or "f32" table/output dtype on device
N_QUEUES = 4      # SWDGE queues (1..4); sub-gathers round-robin over them
                  # (4x faster than 1: parallel ring drain, measured)
M_SUB = 896       # indices per dma_gather (ring limit is 1024; 896 = m/4
                  # keeps the 4 SWDGE queues evenly loaded, measured best)

_CACHE = {}


def host_prep(idx_core: np.ndarray, m: int):
    """Bucket+sort one core's indices. Returns (idx16 [P, N_CHUNKS*m//16],
    dram_row [n] inverse map into the gout tensor)."""
    n = idx_core.shape[0]
    order = np.argsort(idx_core, kind="stable")  # sorts by (chunk, low)
    sidx = idx_core[order]
    chunk = sidx // CHUNK
    low = (sidx - chunk * CHUNK).astype(np.int16)
    counts = np.bincount(chunk, minlength=N_CHUNKS)
    starts = np.concatenate([[0], np.cumsum(counts)[:-1]])
    within = np.arange(n) - starts[chunk]          # slot j within bucket
    idx16 = np.zeros((N_CHUNKS, m), dtype=np.int16)  # pad = dummy row 0
    idx16[chunk, within] = low
    s = m // P
    dram_row = np.empty(n, dtype=np.int64)
    dram_row[order] = chunk * m + (within % P) * s + within // P
    # wrap: index i at partition i%16, position i//16 (per chunk), then
    # replicate the 16-partition block to 128 partitions
    wrapped = (
        idx16.reshape(N_CHUNKS, m // 16, 16)
        .transpose(2, 0, 1)
        .reshape(16, N_CHUNKS * (m // 16))
    )
    return np.tile(wrapped, (8, 1)), dram_row


def _build_bass(m: int, bufs: int, dtype: str, n_queues: int, m_sub: int = M_SUB):
    import concourse.bacc as bacc
    import concourse.mybir as mybir
    import concourse.tile as tile
    from concourse import library_config

    key = (m, bufs, dtype, n_queues, m_sub)
    if key in _CACHE:
        return _CACHE[key]

    bdt = mybir.dt.bfloat16 if dtype == "bf16" else mybir.dt.float32
    s = m // P

    nc = bacc.Bacc(
        "TRN2",
        target_bir_lowering=False,
        debug=False,
        num_devices=N_CORES,
        num_swdge_queues=n_queues,
    )
    idx16_d = nc.dram_tensor(
        "idx16", [P, N_CHUNKS * (m // 16)], mybir.dt.int16, kind="ExternalInput"
    )
    weight = nc.dram_tensor("weight", [NUM_EMB, D], bdt, kind="ExternalInput")
    gout = nc.dram_tensor("gout", [N_CHUNKS * m, D], bdt, kind="ExternalOutput")

    with tile.TileContext(nc) as tc:
        with (
            tc.tile_pool(name="idxp", bufs=1) as idxp,
            tc.tile_pool(name="data", bufs=bufs) as datap,
        ):
            nc.gpsimd.load_library(library_config.mlp)
            idx_tile = idxp.tile([P, N_CHUNKS * (m // 16)], mybir.dt.int16)
            nc.sync.dma_start(idx_tile[:], idx16_d[:])
            gout_r = gout[:].rearrange("(c p s) d -> c p (s d)", c=N_CHUNKS, p=P)
            q = 0
            for c in range(N_CHUNKS):
                hi = min((c + 1) * CHUNK, NUM_EMB)
                dtile = datap.tile([P, s, D], bdt)
                # descriptor-ring limit: <= m_sub indices per dma_gather;
                # sub-gather g fills slots [g*m_sub/P, ...) of the tile, and
                # index i of sub-gather g lands at dtile[i%P][g*m_sub/P+i/P]
                # == absolute slot j//P for j = g*m_sub+i, same as one big
                # gather would
                for g in range(0, m, m_sub):
                    n_sub = min(m_sub, m - g)
                    nc.gpsimd.dma_gather(
                        dtile[:, g // P : (g + n_sub) // P, :],
                        weight[c * CHUNK : hi],
                        idx_tile[:, (c * m + g) // 16 : (c * m + g + n_sub) // 16],
                        n_sub,
                        n_sub,
                        D,
                        queue_num=q % n_queues,
                        single_packet=False,
                    )
                    q += 1
                # alternate the two HWDGE rings (SP / Act) for the stores
                eng = nc.scalar if c % 2 else nc.sync
                eng.dma_start(gout_r[c], dtile[:].rearrange("p s d -> p (s d)"))
    nc.compile()
    _CACHE[key] = nc
    return nc


def run_sharded(indices: np.ndarray, weight: np.ndarray, trace: bool = False):
    """Shard indices across 8 cores, run the Bass kernel, return
    (full_output, BassKernelResults)."""
    from concourse.bass_utils import run_bass_kernel_spmd

    idx_flat = np.ascontiguousarray(indices.reshape(-1).astype(np.int64))
    if DTYPE == "bf16":
        w = np.ascontiguousarray(weight.astype(ml_dtypes.bfloat16))
    else:
        w = np.ascontiguousarray(weight, dtype=np.float32)
    n_idx = idx_flat.shape[0]
    per_core = n_idx // N_CORES
    assert n_idx == per_core * N_CORES

    # static padded bucket capacity shared by all cores (SPMD): max count
    # over all (core, chunk) buckets, rounded up to a full 128-slot multiple
    chunk_ids = idx_flat // CHUNK
    max_count = max(
        int(np.bincount(chunk_ids[c * per_core : (c + 1) * per_core],
                        minlength=N_CHUNKS).max())
        for c in range(N_CORES)
    )
    m = max(M_FLOOR, -(-max_count // P) * P)

    preps = [
        host_prep(idx_flat[c * per_core : (c + 1) * per_core], m)
        for c in range(N_CORES)
    ]

    nc = _build_bass(m, BUFS, DTYPE, N_QUEUES)
    in_maps = [{"idx16": preps[c][0], "weight": w} for c in range(N_CORES)]
    res = run_bass_kernel_spmd(
        nc, in_maps, core_ids=list(range(N_CORES)), trace=trace
    )
    full = np.empty((n_idx, D), dtype=np.float32)
    for c in range(N_CORES):
        gout = np.asarray(res.results[c]["gout"])
        full[c * per_core : (c + 1) * per_core] = gout[preps[c][1]].astype(np.float32)
    return full.reshape(indices.shape + (D,)), res


def kernel(indices: np.ndarray, weight: np.ndarray) -> np.ndarray:
    full, _ = run_sharded(indices, weight, trace=False)
    return full


# revision 12
# speedup vs baseline: 3.1299x; 1.0007x over previous
"""Embedding lookup (weight[indices]) on 8 TRN2 NeuronCores.

Strategy: replicate the 1M x 128 table in each core's HBM (cast to bf16
host-side; rel err ~2^-9 << 2e-2 tolerance), shard the 4096*200 = 819200
indices 8 ways (data parallel).

The SWDGE indirect-DMA path costs ~1 us of Pool-engine descriptor-gen
per instruction and HW only honours ONE offset per partition per
instruction (measured), so gathering row-by-row is fixed-cost bound.
Instead each core uses the InstDMAGatherAnt ucode gather (`dma_gather`),
which takes thousands of int16 indices per instruction: the host sorts
each core's indices, buckets them into 31 table chunks of <=32768 rows
(so low 15 bits fit int16), pads each bucket to a static M with dummy
index 0, and builds the 16-partition-wrapped replicated index tensor.
The device runs one dma_gather + one contiguous store per chunk; the
host inverts the sort/bucket permutation on the bf16 output and upcasts
to f32.  Sorting also makes the gather's HBM reads near-sequential.
"""

import numpy as np
import ml_dtypes

NUM_EMB = 1_000_000
D = 128
N_CORES = 8
P = 128

N_CHUNKS = 31
CHUNK = -(-NUM_EMB // N_CHUNKS)  # 32259 <= 32768 so low bits fit int16
M_FLOOR = 3584                   # per-chunk padded capacity floor (28 slots)

# tuning knobs
BUFS = 4          # SBUF tile double-buffering depth
DTYPE = "bf16"    # "bf16" or "f32" table/output dtype on device
N_QUEUES = 4      # SWDGE queues (1..4); sub-gathers round-robin over them
                  # (4x faster than 1: parallel ring drain, measured)
M_SUB = 896       # indices per dma_gather (ring limit is 1024; 896 = m/4
                  # keeps the 4 SWDGE queues evenly loaded, measured best)

_CACHE = {}


def host_prep(idx_core: np.ndarray, m: int):
    """Dedup+bucket one core's indices. Returns (idx16 [P, N_CHUNKS*m//16],
    dram_row [n] inverse map into the gout tensor).  Only unique rows are
    gathered (the gather is descriptor-rate bound, so ~5% duplicate draws
    are pure waste); duplicates resolve through the same inverse map."""
    uniq = np.unique(idx_core)                     # sorted unique rows
    chunk = uniq // CHUNK
    low = (uniq - chunk * CHUNK).astype(np.int16)
    counts = np.bincount(chunk, minlength=N_CHUNKS)
    starts = np.concatenate([[0], np.cumsum(counts)[:-1]])
    within = np.arange(uniq.size) - starts[chunk]  # slot j within bucket
    idx16 = np.zeros((N_CHUNKS, m), dtype=np.int16)  # pad = dummy row 0
    idx16[chunk, within] = low
    s = m // P
    dram_row_uniq = chunk * m + (within % P) * s + within // P
    dram_row = dram_row_uniq[np.searchsorted(uniq, idx_core)]
    # wrap: index i at partition i%16, position i//16 (per chunk), then
    # replicate the 16-partition block to 128 partitions
    wrapped = (
        idx16.reshape(N_CHUNKS, m // 16, 16)
        .transpose(2, 0, 1)
        .reshape(16, N_CHUNKS * (m // 16))
    )
    return np.tile(wrapped, (8, 1)), dram_row


def compute_m(idx_flat: np.ndarray, per_core: int) -> int:
    """Static padded bucket capacity shared by all cores (SPMD): max count
    over all (core, chunk) DEDUPED buckets, rounded up to 128 slots."""
    max_count = max(
        int(np.bincount(np.unique(idx_flat[c * per_core : (c + 1) * per_core])
                        // CHUNK, minlength=N_CHUNKS).max())
        for c in range(N_CORES)
    )
    return max(P, -(-max_count // P) * P)


def _build_bass(m: int, bufs: int, dtype: str, n_queues: int, m_sub: int = M_SUB):
    import concourse.bacc as bacc
    import concourse.mybir as mybir
    import concourse.tile as tile
    from concourse import library_config

    key = (m, bufs, dtype, n_queues, m_sub)
    if key in _CACHE:
        return _CACHE[key]

    bdt = mybir.dt.bfloat16 if dtype == "bf16" else mybir.dt.float32
    s = m // P

    nc = bacc.Bacc(
        "TRN2",
        target_bir_lowering=False,
        debug=False,
        num_devices=N_CORES,
        num_swdge_queues=n_queues,
    )
    idx16_d = nc.dram_tensor(
        "idx16", [P, N_CHUNKS * (m // 16)], mybir.dt.int16, kind="ExternalInput"
    )
    weight = nc.dram_tensor("weight", [NUM_EMB, D], bdt, kind="ExternalInput")
    gout = nc.dram_tensor("gout", [N_CHUNKS * m, D], bdt, kind="ExternalOutput")

    with tile.TileContext(nc) as tc:
        with (
            tc.tile_pool(name="idxp", bufs=1) as idxp,
            tc.tile_pool(name="data", bufs=bufs) as datap,
        ):
            nc.gpsimd.load_library(library_config.mlp)
            idx_tile = idxp.tile([P, N_CHUNKS * (m // 16)], mybir.dt.int16)
            nc.sync.dma_start(idx_tile[:], idx16_d[:])
            gout_r = gout[:].rearrange("(c p s) d -> c p (s d)", c=N_CHUNKS, p=P)
            q = 0
            for c in range(N_CHUNKS):
                hi = min((c + 1) * CHUNK, NUM_EMB)
                dtile = datap.tile([P, s, D], bdt)
                # descriptor-ring limit: <= m_sub indices per dma_gather;
                # sub-gather g fills slots [g*m_sub/P, ...) of the tile, and
                # index i of sub-gather g lands at dtile[i%P][g*m_sub/P+i/P]
                # == absolute slot j//P for j = g*m_sub+i, same as one big
                # gather would
                for g in range(0, m, m_sub):
                    n_sub = min(m_sub, m - g)
                    nc.gpsimd.dma_gather(
                        dtile[:, g // P : (g + n_sub) // P, :],
                        weight[c * CHUNK : hi],
                        idx_tile[:, (c * m + g) // 16 : (c * m + g + n_sub) // 16],
                        n_sub,
                        n_sub,
                        D,
                        queue_num=q % n_queues,
                        single_packet=False,
                    )
                    q += 1
                # alternate the two HWDGE rings (SP / Act) for the stores
                eng = nc.scalar if c % 2 else nc.sync
                eng.dma_start(gout_r[c], dtile[:].rearrange("p s d -> p (s d)"))
    nc.compile()
    _CACHE[key] = nc
    return nc


def run_sharded(indices: np.ndarray, weight: np.ndarray, trace: bool = False):
    """Shard indices across 8 cores, run the Bass kernel, return
    (full_output, BassKernelResults)."""
    from concourse.bass_utils import run_bass_kernel_spmd

    idx_flat = np.ascontiguousarray(indices.reshape(-1).astype(np.int64))
    if DTYPE == "bf16":
        w = np.ascontiguousarray(weight.astype(ml_dtypes.bfloat16))
    else:
        w = np.ascontiguousarray(weight, dtype=np.float32)
    n_idx = idx_flat.shape[0]
    per_core = n_idx // N_CORES
    assert n_idx == per_core * N_CORES

    m = compute_m(idx_flat, per_core)

    preps = [
        host_prep(idx_flat[c * per_core : (c + 1) * per_core], m)
        for c in range(N_CORES)
    ]

    nc = _build_bass(m, BUFS, DTYPE, N_QUEUES)
    in_maps = [{"idx16": preps[c][0], "weight": w} for c in range(N_CORES)]
    res = run_bass_kernel_spmd(
        nc, in_maps, core_ids=list(range(N_CORES)), trace=trace
    )
    full = np.empty((n_idx, D), dtype=np.float32)
    for c in range(N_CORES):
        gout = np.asarray(res.results[c]["gout"])
        full[c * per_core : (c + 1) * per_core] = gout[preps[c][1]].astype(np.float32)
    return full.reshape(indices.shape + (D,)), res


def kernel(indices: np.ndarray, weight: np.ndarray) -> np.ndarray:
    full, _ = run_sharded(indices, weight, trace=False)
    return full


# revision 14
# speedup vs baseline: 3.3901x; 1.0831x over previous
"""Embedding lookup (weight[indices]) on 8 TRN2 NeuronCores.

Strategy: replicate the 1M x 128 table in each core's HBM (cast to bf16
host-side; rel err ~2^-9 << 2e-2 tolerance), shard the 4096*200 = 819200
indices 8 ways (data parallel).

The SWDGE indirect-DMA path costs ~1 us of Pool-engine descriptor-gen
per instruction and HW only honours ONE offset per partition per
instruction (measured), so gathering row-by-row is fixed-cost bound.
Instead each core uses the InstDMAGatherAnt ucode gather (`dma_gather`),
which takes thousands of int16 indices per instruction: the host sorts
each core's indices, buckets them into 31 table chunks of <=32768 rows
(so low 15 bits fit int16), pads each bucket to a static M with dummy
index 0, and builds the 16-partition-wrapped replicated index tensor.
The device runs one dma_gather + one contiguous store per chunk; the
host inverts the sort/bucket permutation on the bf16 output and upcasts
to f32.  Sorting also makes the gather's HBM reads near-sequential.
"""

import numpy as np
import ml_dtypes

NUM_EMB = 1_000_000
D = 128
N_CORES = 8
P = 128

N_CHUNKS = 31
CHUNK = -(-NUM_EMB // N_CHUNKS)  # 32259 <= 32768 so low bits fit int16
M_FLOOR = 3584                   # per-chunk padded capacity floor (28 slots)

# tuning knobs
BUFS = 4          # SBUF tile double-buffering depth
DTYPE = "bf16"    # "bf16" or "f32" table/output dtype on device
N_QUEUES = 4      # SWDGE queues (1..4); sub-gathers round-robin over them
                  # (4x faster than 1: parallel ring drain, measured)
M_SUB = 896       # indices per dma_gather (ring limit is 1024; 896 = m/4
                  # keeps the 4 SWDGE queues evenly loaded, measured best)

_CACHE = {}


def host_prep(idx_core: np.ndarray, m: int):
    """Dedup+bucket one core's indices. Returns (idx16 [P, N_CHUNKS*m//16],
    dram_row [n] inverse map into the gout tensor).  Only unique rows are
    gathered (the gather is descriptor-rate bound, so ~5% duplicate draws
    are pure waste); duplicates resolve through the same inverse map."""
    uniq = np.unique(idx_core)                     # sorted unique rows
    chunk = uniq // CHUNK
    low = (uniq - chunk * CHUNK).astype(np.int16)
    counts = np.bincount(chunk, minlength=N_CHUNKS)
    starts = np.concatenate([[0], np.cumsum(counts)[:-1]])
    within = np.arange(uniq.size) - starts[chunk]  # slot j within bucket
    idx16 = np.zeros((N_CHUNKS, m), dtype=np.int16)  # pad = dummy row 0
    idx16[chunk, within] = low
    s = m // P
    dram_row_uniq = chunk * m + (within % P) * s + within // P
    dram_row = dram_row_uniq[np.searchsorted(uniq, idx_core)]
    # wrap: index i at partition i%16, position i//16 (per chunk), then
    # replicate the 16-partition block to 128 partitions
    wrapped = (
        idx16.reshape(N_CHUNKS, m // 16, 16)
        .transpose(2, 0, 1)
        .reshape(16, N_CHUNKS * (m // 16))
    )
    return np.tile(wrapped, (8, 1)), dram_row


def compute_m(idx_flat: np.ndarray, per_core: int) -> int:
    """Static padded bucket capacity shared by all cores (SPMD): max count
    over all (core, chunk) DEDUPED buckets, rounded up to 128 slots."""
    max_count = max(
        int(np.bincount(np.unique(idx_flat[c * per_core : (c + 1) * per_core])
                        // CHUNK, minlength=N_CHUNKS).max())
        for c in range(N_CORES)
    )
    return max(P, -(-max_count // P) * P)


def _build_bass(m: int, bufs: int, dtype: str, n_queues: int, m_sub: int = M_SUB):
    import concourse.bacc as bacc
    import concourse.mybir as mybir
    import concourse.tile as tile
    from concourse import library_config

    key = (m, bufs, dtype, n_queues, m_sub)
    if key in _CACHE:
        return _CACHE[key]

    bdt = mybir.dt.bfloat16 if dtype == "bf16" else mybir.dt.float32
    s = m // P

    nc = bacc.Bacc(
        "TRN2",
        target_bir_lowering=False,
        debug=False,
        num_devices=N_CORES,
        num_swdge_queues=n_queues,
    )
    idx16_d = nc.dram_tensor(
        "idx16", [P, N_CHUNKS * (m // 16)], mybir.dt.int16, kind="ExternalInput"
    )
    weight = nc.dram_tensor("weight", [NUM_EMB, D], bdt, kind="ExternalInput")
    gout = nc.dram_tensor("gout", [N_CHUNKS * m, D], bdt, kind="ExternalOutput")

    with tile.TileContext(nc) as tc:
        with (
            tc.tile_pool(name="idxp", bufs=1) as idxp,
            tc.tile_pool(name="data", bufs=bufs) as datap,
        ):
            nc.gpsimd.load_library(library_config.mlp)
            idx_tile = idxp.tile([P, N_CHUNKS * (m // 16)], mybir.dt.int16)
            nc.sync.dma_start(idx_tile[:], idx16_d[:])
            gout_r = gout[:].rearrange("(c p s) d -> c p (s d)", c=N_CHUNKS, p=P)
            for c in range(N_CHUNKS):
                hi = min((c + 1) * CHUNK, NUM_EMB)
                dtile = datap.tile([P, s, D], bdt)
                # descriptor-ring limit: <= m_sub indices per dma_gather;
                # sub-gather g fills slots [g*m_sub/P, ...) of the tile, and
                # index i of sub-gather g lands at dtile[i%P][g*m_sub/P+i/P]
                # == absolute slot j//P for j = g*m_sub+i, same as one big
                # gather would
                for g in range(0, m, m_sub):
                    n_sub = min(m_sub, m - g)
                    nc.gpsimd.dma_gather(
                        dtile[:, g // P : (g + n_sub) // P, :],
                        weight[c * CHUNK : hi],
                        idx_tile[:, (c * m + g) // 16 : (c * m + g + n_sub) // 16],
                        n_sub,
                        n_sub,
                        D,
                        # rotate the starting queue per chunk so the short
                        # tail sub-gather doesn't always land on queue 3 —
                        # per-queue descriptor load stays globally balanced
                        queue_num=(c + g // m_sub) % n_queues,
                        single_packet=False,
                    )
                # alternate the two HWDGE rings (SP / Act) for the stores
                eng = nc.scalar if c % 2 else nc.sync
                eng.dma_start(gout_r[c], dtile[:].rearrange("p s d -> p (s d)"))
    nc.compile()
    _CACHE[key] = nc
    return nc


def run_sharded(indices: np.ndarray, weight: np.ndarray, trace: bool = False):
    """Shard indices across 8 cores, run the Bass kernel, return
    (full_output, BassKernelResults)."""
    from concourse.bass_utils import run_bass_kernel_spmd

    idx_flat = np.ascontiguousarray(indices.reshape(-1).astype(np.int64))
    if DTYPE == "bf16":
        w = np.ascontiguousarray(weight.astype(ml_dtypes.bfloat16))
    else:
        w = np.ascontiguousarray(weight, dtype=np.float32)
    n_idx = idx_flat.shape[0]
    per_core = n_idx // N_CORES
    assert n_idx == per_core * N_CORES

    m = compute_m(idx_flat, per_core)

    preps = [
        host_prep(idx_flat[c * per_core : (c + 1) * per_core], m)
        for c in range(N_CORES)
    ]

    nc = _build_bass(m, BUFS, DTYPE, N_QUEUES)
    in_maps = [{"idx16": preps[c][0], "weight": w} for c in range(N_CORES)]
    res = run_bass_kernel_spmd(
        nc, in_maps, core_ids=list(range(N_CORES)), trace=trace
    )
    full = np.empty((n_idx, D), dtype=np.float32)
    for c in range(N_CORES):
        gout = np.asarray(res.results[c]["gout"])
        full[c * per_core : (c + 1) * per_core] = gout[preps[c][1]].astype(np.float32)
    return full.reshape(indices.shape + (D,)), res


def kernel(indices: np.ndarray, weight: np.ndarray) -> np.ndarray:
    full, _ = run_sharded(indices, weight, trace=False)
    return full
